# revision 34
# baseline (speedup 1.0000x reference)
"""Trainium2 Bass kernel for ContactDiffusion GNN message passing (v3).

out = latent + K_norm @ msg,  K = (D+eps)^(-alpha_ij) * exp(-D/12), row-normalized,
msg = MLP(latent).

Strategy (8 NeuronCores, SPMD single program, full inputs in / full output out):
 - Host: KD-sort points spatially; core c owns 1024 contiguous sorted rows.
 - Per core: K^T slab [8192 x 1024] via fp16-split Gram matmul (exact to
   ~1e-5 abs in d2); elementwise chain split across engines:
     ScalarE: Ln, Exp(d12), Exp(final)  (act-table pass rewrites the stock
              table loads to keep ln+exp resident in one set: 37 -> 5 loads)
     VectorE: fused scalar_tensor_tensor alpha add+mult, t-add, per-tile
              column sums of K (row sums recovered on host via K symmetry)
     GpSimd:  diagonal zeroing (affine_select)
 - All pairs with d2 < TSTRAG (symmetric global set V*) are suppressed on
   device via a rank-1 indicator feature and added back exactly on host.
 - Own-block (8 extra j-tiles per core) computed un-suppressed with the same
   Gram pass; diagonal zeroed by affine_select.
 - MLP sharded per core; msg exchanged via 4 chunked AllGathers issued as
   early as possible and overlapped with own-block elementwise work.
"""

import math
import os
import sys
import types
from contextlib import ExitStack

import numpy as np

sys.path.insert(0, "/opt/trn_rl_repo")

import ml_dtypes

import concourse.bass as bass
import concourse.tile as tile
from concourse import bacc, mybir
from concourse.bass_utils import run_bass_kernel_spmd

F32 = mybir.dt.float32
F16 = mybir.dt.float16
BF16 = mybir.dt.bfloat16
AF = mybir.ActivationFunctionType
ALU = mybir.AluOpType

NP_BF16 = ml_dtypes.bfloat16

N, DIM, NCORE = 8192, 512, 8
NSH = N // NCORE            # rows per core (1024)
EPS, LAM = 1e-4, 12.0
TSTRAG = 0.25               # d2 below this -> straggler (host-fixed exactly)
SUP = 1e3                   # suppressor feature magnitude (SUP^2 added to d2)
LN6 = math.log(6.0)
NT = 64                     # global j-tiles
NOWN = 8                    # own-block j-tiles
NIC = NSH // 128            # i-chunks (8)
NKD = DIM // 128            # MLP k-blocks (4)
NATLOG_EXP_SET = 6          # act_info.json index of natural_log_exp_and_others
GELU_SET = 10               # act_info.json index of gelu_and_others

_BUILT = {}


# ----------------------------------------------------------------------------
# activation-table pass: keep ln+exp resident (set 6); switch to the gelu set
# only around the (grouped) gelu calls.  Replaces the stock pass, which picks
# a table set per-function greedily and reloads on every ln<->exp transition
# (37 loads x 1.28us observed on the scalar critical path).
# ----------------------------------------------------------------------------
def _act_table_pass(self):
    """Run the stock table-load insertion, then rewrite: prefer set 6
    (natural_log_exp_and_others) so ln<->exp transitions don't reload, and
    drop loads made redundant by that choice.  Only mutates the set-id field
    of stock-created instructions / removes sync-free instructions."""
    type(self).insert_act_table_loads(self)

    from concourse.hw_specs import get_activation_tables

    tables = list(get_activation_tables(self.m.arch).items())

    def funcs_of(i):
        return tables[i][1]

    def preferred(f):
        if f in funcs_of(NATLOG_EXP_SET):
            return NATLOG_EXP_SET
        if f in funcs_of(GELU_SET):
            return GELU_SET
        return next(j for j in range(len(tables)) if f in funcs_of(j))

    for blk in self.main_func.blocks:
        cur = None
        pending = None
        drops = []
        for inst in blk.instructions:
            if isinstance(inst, mybir.InstLoadActFuncSet):
                if inst.sync_info is not None:
                    continue  # don't touch synced loads
                if pending is not None:
                    drops.append(pending)
                pending = inst
            elif (isinstance(inst, mybir.InstActivation)
                  and inst.engine == mybir.EngineType.Activation):
                f = inst.func
                if cur is not None and f in funcs_of(cur):
                    if pending is not None:
                        drops.append(pending)
                        pending = None
                else:
                    assert pending is not None, \
                        f"activation {inst.name} uncovered by stock pass"
                    pending.act_func_set_id = preferred(f)
                    cur = pending.act_func_set_id
                    pending = None
        if pending is not None:
            drops.append(pending)
        if drops:
            dropset = set(id(d) for d in drops)
            keep = [i for i in blk.instructions if id(i) not in dropset]
            del blk.instructions[:]
            for i in keep:
                blk.instructions.append(i)


# ----------------------------------------------------------------------------
# device program (single SPMD program for all 8 cores)
# ----------------------------------------------------------------------------
def build_program():
    nc = bacc.Bacc("TRN2", target_bir_lowering=False, debug=False,
                   num_devices=NCORE)
    if os.environ.get("NO_ACT_PASS", "0") != "1":
        nc.insert_act_table_loads = types.MethodType(_act_table_pass, nc)

    featj = nc.dram_tensor("featj", [18, N], F16, kind="ExternalInput").ap()
    feati = nc.dram_tensor("feati", [18, NSH], F16, kind="ExternalInput").ap()
    featjo = nc.dram_tensor("featjo", [18, NSH], F16, kind="ExternalInput").ap()
    ahj = nc.dram_tensor("ahj", [128, NT], F32, kind="ExternalInput").ap()
    ahjo = nc.dram_tensor("ahjo", [128, NOWN], F32, kind="ExternalInput").ap()
    ahibc = nc.dram_tensor("ahibc", [128, NSH], F16, kind="ExternalInput").ap()
    latT = nc.dram_tensor("latT", [DIM, NSH], F16, kind="ExternalInput").ap()
    w1t = nc.dram_tensor("w1t", [DIM, DIM], F16, kind="ExternalInput").ap()
    w2t = nc.dram_tensor("w2t", [DIM, DIM], F16, kind="ExternalInput").ap()
    b1c = nc.dram_tensor("b1c", [128, NKD], F32, kind="ExternalInput").ap()
    b2r = nc.dram_tensor("b2r", [1, DIM], F16, kind="ExternalInput").ap()
    onescol = nc.dram_tensor("onescol", [1, 128], F16, kind="ExternalInput").ap()

    num_out = nc.dram_tensor("num", [NSH, DIM], F32,
                             kind="ExternalOutput").ap()
    colsum_out = nc.dram_tensor("colsum", [128, NOWN + NT], F32,
                                kind="ExternalOutput").ap()

    with tile.TileContext(nc) as tc, ExitStack() as ctx:
        pers = ctx.enter_context(tc.tile_pool(name="pers", bufs=1))
        p_big = ctx.enter_context(tc.tile_pool(name="pbig", bufs=2, space="PSUM"))
        p_out = ctx.enter_context(tc.tile_pool(name="pout", bufs=4, space="PSUM"))
        l_pool = ctx.enter_context(tc.tile_pool(name="lp", bufs=9))
        d12_pool = ctx.enter_context(tc.tile_pool(name="d12", bufs=4))
        m_pool = ctx.enter_context(tc.tile_pool(name="mp", bufs=2))
        tt_pool = ctx.enter_context(tc.tile_pool(name="tt", bufs=3))
        kraw_pool = ctx.enter_context(tc.tile_pool(name="kraw", bufs=1))
        cs_pool = ctx.enter_context(tc.tile_pool(name="cs", bufs=2))
        cs2_pool = ctx.enter_context(tc.tile_pool(name="cs2", bufs=2))
        k_pool = ctx.enter_context(tc.tile_pool(name="kp", bufs=10))
        msg_pool = ctx.enter_context(tc.tile_pool(name="msgp", bufs=16))
        dram = ctx.enter_context(tc.tile_pool(name="dram", bufs=1, space="DRAM"))

        dma = nc.sync.dma_start

        def load_set(set_id):
            ld = mybir.InstLoadActFuncSet(
                name=nc.get_next_instruction_name(), ins=[], outs=[],
                act_func_set_id=set_id)
            ld.engine = mybir.EngineType.Activation
            nc.scalar.add_instruction(ld)

        # ---- persistent SBUF loads (phase-critical first) ----
        featjo_sb = pers.tile([18, NSH], F16)
        dma(featjo_sb[:], featjo[:])
        feati_sb = pers.tile([18, NSH], F16)
        dma(feati_sb[:], feati[:])
        # whole featj resident (16KB/partition): per-tile feature DMAs
        # otherwise serialize behind AllGather-gated msg loads on shared
        # DMA-queue semaphores, stalling the gram pipeline ~33us (measured).
        featj_sb = pers.tile([18, N], F16)
        dma(featj_sb[:], featj[:])
        w1t_sb = [pers.tile([128, DIM], F16, tag=f"w1t{k}", name=f"w1t{k}")
                  for k in range(NKD)]
        for k in range(NKD):
            dma(w1t_sb[k][:], w1t[k * 128:(k + 1) * 128, :])
        b1c_sb = pers.tile([128, NKD], F32)
        dma(b1c_sb[:], b1c[:])
        lat_sb = [pers.tile([128, NSH], F16, tag=f"lat{k}", name=f"lat{k}")
                  for k in range(NKD)]
        for k in range(NKD):
            dma(lat_sb[k][:], latT[k * 128:(k + 1) * 128, :])
        ahibc_sb = pers.tile([128, NSH], F16)
        dma(ahibc_sb[:], ahibc[:])
        ahjo_sb = pers.tile([128, NOWN], F32)
        dma(ahjo_sb[:], ahjo[:])
        w2t_sb = [pers.tile([128, DIM], F16, tag=f"w2t{k}", name=f"w2t{k}")
                  for k in range(NKD)]
        for k in range(NKD):
            dma(w2t_sb[k][:], w2t[k * 128:(k + 1) * 128, :])
        b2r_sb = pers.tile([1, DIM], F16)
        dma(b2r_sb[:], b2r[:])
        onescol_sb = pers.tile([1, 128], F16)
        dma(onescol_sb[:], onescol[:])
        ahj_sb = pers.tile([128, NT], F32)
        dma(ahj_sb[:], ahj[:])

        acc = pers.tile([128, NIC * DIM], F32)
        colsum_sb = pers.tile([128, NOWN + NT], F32)
        bias_ln6 = pers.tile([128, 1], F32)
        nc.gpsimd.memset(bias_ln6[:], -LN6)

        hT_sb = [pers.tile([128, NSH], F16, tag=f"hT{k}", name=f"hT{k}")
                 for k in range(NKD)]
        msgown_sb = [pers.tile([128, DIM], BF16, tag=f"mo{ic}", name=f"mo{ic}")
                     for ic in range(NIC)]
        msgown_ch = [dram.tile([2 * 128, DIM], BF16, tag=f"moch{c}",
                               name=f"moch{c}") for c in range(4)]
        msgall_ch = [dram.tile([16 * 128, DIM], BF16, tag=f"mach{c}",
                               name=f"mach{c}") for c in range(4)]

        # ---- helpers (pair-merged elementwise at FD=2048) ----
        def gram(lhs_sb, col0):
            pd2 = p_big.tile([128, NSH], F32, tag="big", name="pd2")
            for half in range(2):
                hs = slice(half * 512, (half + 1) * 512)
                nc.tensor.matmul(pd2[:, hs],
                                 lhsT=lhs_sb[:, col0:col0 + 128],
                                 rhs=feati_sb[:, hs],
                                 start=True, stop=True)
            return pd2

        def gt_of(k):
            return (k % 8) * 8 + k // 8

        def gram_global(k):
            gt = gt_of(k)
            return gram(featj_sb, gt * 128)

        def ln_to(l2, sub, pd2):
            nc.scalar.activation(l2[:, sub * NSH:(sub + 1) * NSH], pd2[:],
                                 AF.Ln)

        def pair_chain(l2, ah_cols, name, pool=None):
            """d12/fused-alpha/t/final-exp on a [128, 2*NSH] pair"""
            pool = pool or k_pool
            d12 = d12_pool.tile([128, 2 * NSH], F16, tag="d12",
                                name=f"d12{name}")
            nc.scalar.activation(d12[:], l2[:], AF.Exp,
                                 bias=bias_ln6[:, 0:1], scale=0.5)
            m = m_pool.tile([128, 2 * NSH], F16, tag="m", name=f"m{name}")
            if os.environ.get("NO_STT", "0") == "1":
                al = m_pool.tile([128, 2 * NSH], F16, tag="al",
                                 name=f"al{name}")
                for sub in range(2):
                    sl = slice(sub * NSH, (sub + 1) * NSH)
                    nc.vector.tensor_scalar_add(al[:, sl], ahibc_sb[:],
                                                ah_cols[sub])
                nc.vector.tensor_tensor(m[:], al[:], l2[:], op=ALU.mult)
            else:
                for sub in range(2):
                    sl = slice(sub * NSH, (sub + 1) * NSH)
                    nc.vector.scalar_tensor_tensor(
                        m[:, sl], ahibc_sb[:], ah_cols[sub], l2[:, sl],
                        op0=ALU.add, op1=ALU.mult)
            t = tt_pool.tile([128, 2 * NSH], F16, tag="tt", name=f"tt{name}")
            nc.vector.tensor_tensor(t[:], m[:], d12[:], op=ALU.add)
            kt2 = pool.tile([128, 2 * NSH], BF16, tag="kt", name=f"kt{name}")
            nc.scalar.activation(kt2[:], t[:], AF.Exp, scale=-0.5)
            return kt2

        def colsum(kt2, sub, col_idx):
            # fold halves at 2x (bf16 tensor_tensor) then 1x-reduce half the
            # width: 921ns vs 1209ns per tile on the busiest engine (DVE)
            half = cs_pool.tile([128, NSH // 2], BF16, tag="csh",
                                name=f"csh{col_idx}")
            base = sub * NSH
            nc.vector.tensor_tensor(
                half[:], kt2[:, base:base + NSH // 2],
                kt2[:, base + NSH // 2:base + NSH], op=ALU.add)
            quart = cs2_pool.tile([128, NSH // 4], BF16, tag="csq",
                                  name=f"csq{col_idx}")
            nc.vector.tensor_tensor(
                quart[:], half[:, :NSH // 4], half[:, NSH // 4:],
                op=ALU.add)
            nc.vector.tensor_reduce(
                colsum_sb[:, col_idx:col_idx + 1], quart[:],
                axis=mybir.AxisListType.X, op=ALU.add)

        def contract_mm(group, gidx):
            """group: list of (kt2, sub, msgtile); matmuls into 8 po tiles"""
            pos = []
            for ic in range(NIC):
                po = p_out.tile([128, DIM], F32, tag="out",
                                name=f"po{gidx}_{ic}")
                for i, (kt2, sub, mt) in enumerate(group):
                    c0 = sub * NSH + ic * 128
                    nc.tensor.matmul(
                        po[:], lhsT=kt2[:, c0:c0 + 128], rhs=mt[:],
                        start=(i == 0), stop=(i == len(group) - 1))
                pos.append(po)
            return pos

        def acc_add(pos, ics, first=False):
            for ic in ics:
                asl = slice(ic * DIM, (ic + 1) * DIM)
                if first:
                    nc.vector.tensor_copy(acc[:, asl], pos[ic][:])
                else:
                    nc.vector.tensor_tensor(acc[:, asl], acc[:, asl],
                                            pos[ic][:], op=ALU.add)

        # ---- own pair 0 grams + Ln (scalar warms up immediately) ----
        own_l2 = {p: l_pool.tile([128, 2 * NSH], F16, tag="l2",
                                 name=f"l2own{p}") for p in range(4)}
        own_kt2 = {}
        load_set(NATLOG_EXP_SET)
        for r in range(2):
            ln_to(own_l2[0], r % 2, gram(featjo_sb, r * 128))

        # ---- phase A stage 1: hT = gelu(latent @ W1^T + b1) ----
        load_set(GELU_SET)
        for mc in range(NKD):
            ph = p_big.tile([128, NSH], F32, tag="big", name="ph")
            for half in range(2):
                hs = slice(half * 512, (half + 1) * 512)
                for kb in range(NKD):
                    nc.tensor.matmul(
                        ph[:, hs],
                        lhsT=w1t_sb[kb][:, mc * 128:(mc + 1) * 128],
                        rhs=lat_sb[kb][:, hs],
                        start=(kb == 0), stop=(kb == NKD - 1))
            nc.scalar.activation(hT_sb[mc][:], ph[:], AF.Gelu,
                                 bias=b1c_sb[:, mc:mc + 1], scale=1.0)

        # ---- phase A stage 2: msg = hT^T @ W2^T + b2 ; chunked AllGather ----
        for ic in range(NIC):
            pm = p_out.tile([128, DIM], F32, tag="out", name="pm")
            for kb in range(NKD):
                nc.tensor.matmul(
                    pm[:],
                    lhsT=hT_sb[kb][:, ic * 128:(ic + 1) * 128],
                    rhs=w2t_sb[kb][:],
                    start=(kb == 0), stop=False)
            nc.tensor.matmul(pm[:], lhsT=onescol_sb[:], rhs=b2r_sb[:],
                             start=False, stop=True)
            nc.vector.tensor_copy(msgown_sb[ic][:], pm[:])
            ch = ic // 2
            dma(msgown_ch[ch][(ic % 2) * 128:(ic % 2) * 128 + 128, :],
                msgown_sb[ic][:])
            if ic % 2 == 1:
                nc.gpsimd.collective_compute(
                    "AllGather", ALU.bypass,
                    ins=[msgown_ch[ch].opt()], outs=[msgall_ch[ch].opt()],
                    replica_groups=[list(range(NCORE))])

        # ---- own pairs 1..3 grams + Ln, then own chains (overlap stage-2
        # PE work and the AllGathers with ACT/DVE elementwise) ----
        load_set(NATLOG_EXP_SET)
        for r in range(2, NOWN):
            ln_to(own_l2[r // 2], r % 2, gram(featjo_sb, r * 128))
        for p in range(4):
            kraw2 = pair_chain(own_l2[p],
                               [ahjo_sb[:, 2 * p:2 * p + 1],
                                ahjo_sb[:, 2 * p + 1:2 * p + 2]], f"ow{p}",
                               pool=kraw_pool)
            kt2 = k_pool.tile([128, 2 * NSH], BF16, tag="kt",
                              name=f"ktow{p}")
            own_kt2[p] = kt2
            for sub in range(2):
                r = 2 * p + sub
                nc.gpsimd.affine_select(
                    kt2[:, sub * NSH:(sub + 1) * NSH],
                    kraw2[:, sub * NSH:(sub + 1) * NSH],
                    pattern=[[1, NSH]],
                    compare_op=ALU.not_equal, fill=0.0,
                    base=-(r * 128), channel_multiplier=-1)
            for sub in range(2):
                colsum(own_kt2[p], sub, 2 * p + sub)

        # ---- slab pipeline ----
        glob_kt2 = {}
        glob_mt = {}

        def ew_front(g):
            l2s = {}
            for p in range(8):
                l2s[p] = l_pool.tile([128, 2 * NSH], F16, tag="l2",
                                     name=f"l2g{g}p{p}")
                for sub in range(2):
                    k = g * 16 + 2 * p + sub
                    ln_to(l2s[p], sub, gram_global(k))
            return l2s

        def ew_back_pairs(g, plist, l2s):
            for p in plist:
                k0 = g * 16 + 2 * p
                kt2 = pair_chain(
                    l2s[p],
                    [ahj_sb[:, gt_of(k0):gt_of(k0) + 1],
                     ahj_sb[:, gt_of(k0 + 1):gt_of(k0 + 1) + 1]],
                    f"g{g}p{p}")
                glob_kt2[k0 // 2] = kt2
                for sub in range(2):
                    k = k0 + sub
                    colsum(kt2, sub, NOWN + k)
                    csrc, r = k % 8, k // 8
                    mt = msg_pool.tile([128, DIM], BF16)
                    dma(mt[:], msgall_ch[r // 2][csrc * 256 + (r % 2) * 128:
                                                 csrc * 256 + (r % 2) * 128
                                                 + 128, :])
                    glob_mt[k] = mt

        def gp_entries(g, plist):
            return [(glob_kt2[(g * 16 + 2 * p) // 2], sub,
                     glob_mt[g * 16 + 2 * p + sub])
                    for p in plist for sub in range(2)]

        for g in range(3):
            l2s = ew_front(g)
            if g == 0:
                prev = [(own_kt2[r // 2], r % 2, msgown_sb[r])
                        for r in range(NOWN)]
            else:
                prev = gp_entries(g - 1, range(8))
            pos = contract_mm(prev, g)
            ew_back_pairs(g, range(0, 4), l2s)
            acc_add(pos, range(0, 4), first=(g == 0))
            ew_back_pairs(g, range(4, 8), l2s)
            acc_add(pos, range(4, 8), first=(g == 0))
        # final group: progressively finer contract batches so the tail after
        # the last elementwise chain is just a 4-entry contract + acc
        l2s = ew_front(3)
        pos = contract_mm(gp_entries(2, range(8)), 3)
        ew_back_pairs(3, range(0, 4), l2s)
        acc_add(pos, range(0, 8))
        pos_a = contract_mm(gp_entries(3, range(0, 4)), 4)
        ew_back_pairs(3, range(4, 6), l2s)
        acc_add(pos_a, range(0, 8))
        pos_b = contract_mm(gp_entries(3, range(4, 6)), 5)
        ew_back_pairs(3, range(6, 8), l2s)
        acc_add(pos_b, range(0, 8))
        pos_c = contract_mm(gp_entries(3, range(6, 8)), 6)
        acc_add(pos_c, range(0, 8))

        # ---- epilogue ----
        dma(colsum_out[:], colsum_sb[:])
        for ic in range(NIC):
            dma(num_out[ic * 128:(ic + 1) * 128, :],
                acc[:, ic * DIM:(ic + 1) * DIM])

    nc.compile()
    return nc


# ----------------------------------------------------------------------------
# host-side preprocessing
# ----------------------------------------------------------------------------
def _kdsort(coords, nblocks):
    def rec(idx, nb):
        if nb == 1:
            return [idx]
        pts = coords[idx]
        ax = int(np.argmax(pts.max(0) - pts.min(0)))
        order = np.argsort(pts[:, ax], kind="stable")
        half = len(idx) // 2
        return rec(idx[order[:half]], nb // 2) + rec(idx[order[half:]], nb // 2)

    return np.concatenate(rec(np.arange(coords.shape[0]), nblocks))


def _split16(x64):
    """fp16 hi/lo split of a float64 array (captures ~22 mantissa bits)"""
    hi = x64.astype(np.float16)
    lo = (x64 - hi.astype(np.float64)).astype(np.float16)
    return hi, lo


_erf = np.vectorize(math.erf)


def kernel(latent, coords, alpha, W1, b1, W2, b2):
    latent = np.asarray(latent, np.float32)
    coords = np.asarray(coords, np.float32)
    alpha = np.asarray(alpha, np.float32)
    W1 = np.asarray(W1, np.float32)
    b1 = np.asarray(b1, np.float32)
    W2 = np.asarray(W2, np.float32)
    b2 = np.asarray(b2, np.float32)

    perm = _kdsort(coords.astype(np.float64), 64)
    cs = coords[perm]
    als = alpha[perm]
    lats = latent[perm]
    c64 = cs.astype(np.float64)

    # ---- V*: symmetric global straggler set (all pairs d2 < TSTRAG) ----
    close = set()
    for i0 in range(0, N, 1024):
        blk = c64[i0:i0 + 1024]
        d2b = ((blk[:, None, :] - c64[None, :, :]) ** 2).sum(-1)
        d2b[np.arange(1024), np.arange(i0, i0 + 1024)] = np.inf
        ii, jj = np.nonzero(d2b < TSTRAG)
        close.update((i0 + ii).tolist())
        close.update(jj.tolist())
    VV = np.array(sorted(close), dtype=np.int64)
    g = np.zeros(N, np.float64)
    if len(VV):
        g[VV] = SUP

    # ---- fp16-split features ----
    r64 = (c64 ** 2).sum(-1)
    a64 = -2.0 * c64
    chj = [_split16(c64[:, d]) for d in range(3)]
    ahi = [_split16(a64[:, d]) for d in range(3)]
    rj = _split16(r64)
    ones_n = np.ones(N, np.float16)

    rows_j, rows_i_full = [], []
    for d in range(3):
        for (jp, ip) in [(chj[d][0], ahi[d][0]), (chj[d][0], ahi[d][1]),
                         (chj[d][1], ahi[d][0]), (chj[d][1], ahi[d][1])]:
            rows_j.append(jp)
            rows_i_full.append(ip)
    rows_j += [rj[0], rj[1], ones_n, ones_n, g.astype(np.float16)]
    rows_i_full += [ones_n, ones_n, rj[0], rj[1], g.astype(np.float16)]
    # row 17: mown (per-core) x SUP const
    featj_base = np.stack(rows_j).astype(np.float16)          # [17, N]
    feati_base = np.stack(rows_i_full).astype(np.float16)     # [17, N]

    ah = (als.astype(np.float64) / 2.0)
    ahj_all = ah.reshape(NT, 128).T.astype(np.float32).copy()  # [128, 64]

    in_maps = []
    for core in range(NCORE):
        blk = slice(core * NSH, (core + 1) * NSH)
        mown = np.zeros(N, np.float16)
        mown[blk] = SUP
        featj = np.concatenate([featj_base, mown[None, :]], axis=0)
        feati = np.concatenate(
            [feati_base[:, blk],
             np.full((1, NSH), SUP, np.float16)], axis=0)
        featjo = featj[:, blk].copy()
        featjo[17] = 0.0

        ahjo = ah[blk].reshape(NOWN, 128).T.astype(np.float32).copy()
        ahibc = np.broadcast_to(ah[blk].astype(np.float16),
                                (128, NSH)).copy()

        in_maps.append({
            "featj": np.ascontiguousarray(featj),
            "feati": np.ascontiguousarray(feati),
            "featjo": np.ascontiguousarray(featjo),
            "ahj": ahj_all,
            "ahjo": np.ascontiguousarray(ahjo),
            "ahibc": ahibc,
            "latT": lats[blk].T.astype(np.float16).copy(),
            "w1t": W1.T.astype(np.float16).copy(),
            "w2t": W2.T.astype(np.float16).copy(),
            "b1c": b1.reshape(NKD, 128).T.astype(np.float32).copy(),
            "b2r": b2.reshape(1, DIM).astype(np.float16),
            "onescol": np.ones((1, 128), np.float16),
        })

    if "nc" not in _BUILT:
        _BUILT["nc"] = build_program()
    nc = _BUILT["nc"]
    res = run_bass_kernel_spmd(nc, in_maps, core_ids=list(range(NCORE)))

    num_all = np.zeros((N, DIM), np.float32)
    s_all = np.zeros(N, np.float64)
    for core in range(NCORE):
        blk = slice(core * NSH, (core + 1) * NSH)
        num_all[blk] = res.results[core]["num"]
        colsum = res.results[core]["colsum"].astype(np.float64)  # [128, 72]
        # own tiles r=0..7: global j = core*NSH + r*128 + p
        for r in range(NOWN):
            s_all[core * NSH + r * 128: core * NSH + (r + 1) * 128] += \
                colsum[:, r]
        # global tiles k: gt = (k%8)*8 + k//8 ; j = gt*128 + p
        for k in range(NT):
            gt = (k % 8) * 8 + k // 8
            s_all[gt * 128:(gt + 1) * 128] += colsum[:, NOWN + k]

    # ---- host fix: exact K over the suppressed V* x V* grid ----
    if len(VV):
        lr = lats[VV].astype(np.float64)
        hh = lr @ W1.T.astype(np.float64) + b1.astype(np.float64)
        hh = hh * 0.5 * (1.0 + _erf(hh / np.sqrt(2.0)))
        msgV = hh @ W2.T.astype(np.float64) + b2.astype(np.float64)
        cV = c64[VV]
        d2V = ((cV[:, None, :] - cV[None, :, :]) ** 2).sum(-1)
        DV = np.sqrt(np.maximum(d2V, 0.0))
        aV = (als[VV].astype(np.float64)[:, None]
              + als[VV].astype(np.float64)[None, :]) * 0.5
        KV = (DV + EPS) ** (-aV) * np.exp(-DV / LAM)
        np.fill_diagonal(KV, 0.0)
        s_all[VV] += KV.sum(axis=1)
        num_all[VV] += (KV @ msgV).astype(np.float32)

    out = lats + num_all / (s_all[:, None].astype(np.float32) + 1e-8)
    final = np.empty_like(out)
    final[perm] = out
    return final.astype(np.float32)
